# revision 9
# baseline (speedup 1.0000x reference)
"""Trainium2 Bass kernel for nn_Conv2d_35407710388668.

Math: the reference's einsum("icwh,jcwh->ijwh", x, y)/C followed by a
full-spatial VALID box conv collapses to a single GEMM:

    out[i, j] = (1/C) * sum_{c,w,h} x[i,c,w,h] * y[j,c,w,h] * kern[w,h] + 0.1

with contraction K = C*W*H = 131072, M = N = 128.

Sharding: contraction (channel) dim split across the 8 NeuronCores (64
channels each) -- each core reads only its 1/8 slice of BOTH x and y
(total HBM traffic = inputs read exactly once, which is the floor; the
hinted N1-sharding would replicate y 8x).  Each core computes a partial
[128,128] GEMM; the host sums the 8 partials in f64, scales, adds bias.

Default implementation streams the operands as fp8 e4m3 (halves HBM
traffic vs bf16 -- the stream is the roofline here: ~4.2 MB/core at
~336 B/ns) and contracts with DoubleRow fp8 matmuls (2 k-tiles = 256
contraction rows per PE instruction, ~1.4x bf16 PE throughput, so PE
stays under the DMA).  x and y chunks are packed interleaved in ONE
DRAM image so each chunk-pair is a single DMA; chunks alternate between
the SP and ACT HWDGE rings with tapered sizes so PE starts early and
the post-last-chunk tail is short.

fp8 e4m3 is safe here: the output is 0.1 +- ~0.003 and quantization
noise averages out over the 131072-term dot product (measured ~1e-3
relative error, vs the 2e-2 gate).  The conv kernel is folded into x
normalized by 256 (exactly 1.0/elem for the reference's box kernel) so
x stays ~N(0,1) inside fp8 range; the 1/(C*256) scale and +0.1 bias are
applied on host.  |x| <= ~6 << 240, so OCP e4m3 bit patterns match TRN
float8e4 exactly.

Set KERNEL_IMPL=packed for the older bf16 variant, fp8 for fp8 without
DoubleRow.
"""

import numpy as np
import ml_dtypes


def _ensure_axon_profile_hook():
    """Best-effort: register the NTFF profile hook registry that
    concourse.bass_utils expects under axon when trace is requested.
    The container's antenv package lacks the axon_hooks module; the
    actual ctypes hook implementation ships in trn_agent_boot."""
    import sys
    import types

    try:
        import antenv

        if "antenv.axon_hooks" in sys.modules:
            return
        mod = types.ModuleType("antenv.axon_hooks")
        _state = {"hook": None}
        mod.set_axon_ntff_profile_hook = lambda h: _state.__setitem__("hook", h)
        mod.get_axon_ntff_profile_hook = lambda: _state["hook"]
        sys.modules["antenv.axon_hooks"] = mod
        antenv.axon_hooks = mod
        from trn_agent_boot.trn_boot import _ntff_profile_via_ctypes

        mod.set_axon_ntff_profile_hook(
            _ntff_profile_via_ctypes("/opt/axon/libaxon_pjrt.so")
        )
    except Exception:
        pass


_ensure_axon_profile_hook()

N1 = 128
N2 = 128
C = 512
W = 16
H = 16
NCORES = 8
CPC = C // NCORES        # channels per core = 64
KL = CPC * W * H         # per-core contraction length = 16384
KT = KL // 128           # k-tiles per core = 128
VAR_BIAS = 0.1

_CACHE = {}
LAST_RESULTS = None      # test harness reads exec_time_ns from here

# Chunk sizes in k-tiles (one k-tile = 128 contraction rows).  Tapered:
# small first chunks so PE starts early, big middle chunks for DMA
# efficiency, smaller last chunk so the PE tail after the final arrival
# is short.  All even so DoubleRow 2-ktile pairs never straddle a chunk.
CHUNKS_BF16 = [4, 8, 16, 24, 32, 24, 12, 8]
# PE consumes ~2x faster than DMA supplies, so PE-end = land(last chunk)
# + PE(last chunk); decreasing tail keeps every boundary non-critical
# and both rings (SP: even chunks, ACT: odd) finish together.
CHUNKS_FP8 = [4, 8, 16, 24, 24, 16, 12, 12, 8, 4]
# 124-row k-tiles: SDMA engine 15 (which serves partitions 92-95 and
# 124-127, and runs ~30% slower than engines 0-14 -- it paces the whole
# stream at 128 rows) gets half an engine's share when rows 124-127 are
# never touched.  16384 real contraction rows pad to 134 tiles x 124.
NP = 124
KTP = 134
CHUNKS_FP8_124 = [4, 8, 16, 24, 24, 18, 14, 12, 8, 6]
assert sum(CHUNKS_BF16) == KT and sum(CHUNKS_FP8) == KT
assert sum(CHUNKS_FP8_124) == KTP and all(c % 2 == 0 for c in CHUNKS_FP8_124)


def _starts(chunks):
    return [sum(chunks[:i]) for i in range(len(chunks))]


def _build_bass_packed():
    """bf16 variant (the previous baseline): x and y chunks packed
    interleaved in ONE DRAM image, chunks alternating between the SP and
    ACT HWDGE rings, 128 accumulating bf16 matmuls."""
    import concourse.bass as bass
    import concourse.mybir as mybir

    CHUNKS = CHUNKS_BF16
    STARTS = _starts(CHUNKS)
    NCHK = len(CHUNKS)

    nc = bass.Bass(
        "TRN2", target_bir_lowering=False, debug=False, num_devices=NCORES
    )
    zt = nc.dram_tensor("zt", [128, 2 * KL], mybir.dt.bfloat16, kind="ExternalInput")
    out = nc.dram_tensor("out", [128, 128], mybir.dt.float32, kind="ExternalOutput")

    zbuf = nc.alloc_sbuf_tensor("zbuf", [128, 2 * KL], mybir.dt.bfloat16)
    rbuf = nc.alloc_sbuf_tensor("rbuf", [128, 128], mybir.dt.float32)
    acc = nc.alloc_psum_tensor("acc", [128, 128], mybir.dt.float32)

    def off_x(c):
        return 2 * STARTS[c] * 128

    def off_y(c):
        return off_x(c) + CHUNKS[c] * 128

    import contextlib

    with contextlib.ExitStack() as st:
        csems = [st.enter_context(nc.semaphore(f"cs{i}")) for i in range(NCHK)]
        ms = st.enter_context(nc.semaphore("ms"))
        vs = st.enter_context(nc.semaphore("vs"))
        osem = st.enter_context(nc.semaphore("osem"))
        blk = st.enter_context(contextlib.ExitStack())
        block = blk.enter_context(nc.Block())

        @block.sync
        def _(sync):
            for c in range(0, NCHK, 2):
                s = slice(off_x(c), off_x(c) + 2 * CHUNKS[c] * 128)
                sync.dma_start(zbuf[:, s], zt[:, s]).then_inc(csems[c], 16)
            sync.wait_ge(vs, 1)
            sync.dma_start(out[:], rbuf[:]).then_inc(osem, 16)
            sync.wait_ge(osem, 16)

        @block.scalar
        def _(scalar):
            for c in range(1, NCHK, 2):
                s = slice(off_x(c), off_x(c) + 2 * CHUNKS[c] * 128)
                scalar.dma_start(zbuf[:, s], zt[:, s]).then_inc(csems[c], 16)

        @block.tensor
        def _(tensor):
            t = 0
            for c in range(NCHK):
                tensor.wait_ge(csems[c], 16)
                for tl in range(CHUNKS[c]):
                    mm = tensor.matmul(
                        acc[:],
                        zbuf[:, off_x(c) + tl * 128:off_x(c) + (tl + 1) * 128],
                        zbuf[:, off_y(c) + tl * 128:off_y(c) + (tl + 1) * 128],
                        start=(t == 0),
                        stop=(t == KT - 1),
                    )
                    t += 1
            mm.then_inc(ms)

        @block.vector
        def _(vector):
            vector.wait_ge(ms, 1)
            vector.tensor_copy(rbuf[:], acc[:]).then_inc(vs)

        blk.close()

    return nc


def _build_bass_fp8(double_row=True, npart=128):
    """fp8 e4m3 operands, optionally DoubleRow (2 k-tiles per PE
    instruction).  The z image is 3D [npart, 2*nkt, 128] so a DoubleRow
    lhsT/rhs AP is a plain [:, 2t:2t+2, :] slice."""
    import concourse.bass as bass
    import concourse.mybir as mybir

    CHUNKS = CHUNKS_FP8 if npart == 128 else CHUNKS_FP8_124
    nkt = KT if npart == 128 else KTP
    STARTS = _starts(CHUNKS)
    NCHK = len(CHUNKS)

    nc = bass.Bass(
        "TRN2", target_bir_lowering=False, debug=False, num_devices=NCORES
    )
    zt = nc.dram_tensor(
        "zt", [npart, 2 * nkt, 128], mybir.dt.float8e4, kind="ExternalInput"
    )
    out = nc.dram_tensor("out", [128, 128], mybir.dt.bfloat16, kind="ExternalOutput")

    zbuf = nc.alloc_sbuf_tensor("zbuf", [npart, 2 * nkt, 128], mybir.dt.float8e4)
    rbuf = nc.alloc_sbuf_tensor("rbuf", [128, 128], mybir.dt.bfloat16)
    acc = nc.alloc_psum_tensor("acc", [128, 128], mybir.dt.float32)

    def off_x(c):  # packed k-tile index of chunk c's x block
        return 2 * STARTS[c]

    def off_y(c):
        return off_x(c) + CHUNKS[c]

    import contextlib

    with contextlib.ExitStack() as st:
        csems = [st.enter_context(nc.semaphore(f"cs{i}")) for i in range(NCHK)]
        ms = st.enter_context(nc.semaphore("ms"))
        vs = st.enter_context(nc.semaphore("vs"))
        osem = st.enter_context(nc.semaphore("osem"))
        blk = st.enter_context(contextlib.ExitStack())
        block = blk.enter_context(nc.Block())

        @block.sync
        def _(sync):
            for c in range(0, NCHK, 2):
                s = slice(off_x(c), off_x(c) + 2 * CHUNKS[c])
                sync.dma_start(zbuf[:, s, :], zt[:, s, :]).then_inc(csems[c], 16)
            sync.wait_ge(vs, 1)
            sync.dma_start(out[:], rbuf[:]).then_inc(osem, 16)
            sync.wait_ge(osem, 16)

        @block.scalar
        def _(scalar):
            for c in range(1, NCHK, 2):
                s = slice(off_x(c), off_x(c) + 2 * CHUNKS[c])
                scalar.dma_start(zbuf[:, s, :], zt[:, s, :]).then_inc(csems[c], 16)

        @block.tensor
        def _(tensor):
            step = 2 if double_row else 1
            pm = mybir.MatmulPerfMode.DoubleRow if double_row else None
            t = 0
            for c in range(NCHK):
                tensor.wait_ge(csems[c], 16)
                for tl in range(0, CHUNKS[c], step):
                    xs = slice(off_x(c) + tl, off_x(c) + tl + step)
                    ys = slice(off_y(c) + tl, off_y(c) + tl + step)
                    mm = tensor.matmul(
                        acc[:],
                        zbuf[:, xs, :],
                        zbuf[:, ys, :],
                        start=(t == 0),
                        stop=(t + step >= nkt),
                        perf_mode=pm,
                    )
                    t += step
            mm.then_inc(ms)

        @block.vector
        def _(vector):
            vector.wait_ge(ms, 1)
            vector.tensor_copy(rbuf[:], acc[:]).then_inc(vs)

        blk.close()

    return nc


def _sbuf_images(a, np_dt, npart=128):
    """[N, C, W, H] -> [core, p, t, m] images; contraction element
    k = t*npart + p of core c maps to input index (c*CPC*256 + k),
    zero-padded past KL when npart*ntiles > KL."""
    nt = KT if npart == 128 else KTP
    b = a.astype(np_dt).reshape(N1, NCORES, KL)
    if npart * nt != KL:
        pad = np.zeros((N1, NCORES, npart * nt - KL), dtype=np_dt)
        b = np.concatenate([b, pad], axis=2)
    b = b.reshape(N1, NCORES, nt, npart).transpose(1, 3, 2, 0)
    return np.ascontiguousarray(b)  # [NCORES, npart, nt, 128]


def _packed_images(xi, yi, chunks):
    """Interleave per-core x/y images chunkwise into one z image
    [NCORES, npart, 2*ntiles, 128]."""
    starts = _starts(chunks)
    nc_, npart, nt, m = xi.shape
    z = np.empty((nc_, npart, 2 * nt, m), dtype=xi.dtype)
    for s, ch in zip(starts, chunks):
        z[:, :, 2 * s:2 * s + ch] = xi[:, :, s:s + ch]
        z[:, :, 2 * s + ch:2 * s + 2 * ch] = yi[:, :, s:s + ch]
    return z


def kernel(x, y, kernel):
    global LAST_RESULTS
    from concourse import bass_utils

    import os as _os

    impl = _os.environ.get("KERNEL_IMPL", "fp8dr124")
    if "nc" not in _CACHE:
        _CACHE["nc"] = {
            "packed": _build_bass_packed,
            "fp8": lambda: _build_bass_fp8(double_row=False),
            "fp8dr": lambda: _build_bass_fp8(double_row=True),
            "fp8dr124": lambda: _build_bass_fp8(double_row=True, npart=NP),
        }[impl]()
        _CACHE["impl"] = impl
    nc = _CACHE["nc"]
    impl = _CACHE["impl"]

    k2d = np.asarray(kernel, dtype=np.float32).reshape(W, H)
    xf = np.asarray(x, dtype=np.float32) * (k2d * (W * H))  # ~1.0/elem box kernel
    scale = 1.0 / (C * W * H)

    if impl == "packed":
        xi = _sbuf_images(xf, ml_dtypes.bfloat16).reshape(NCORES, 128, KL)
        yi = _sbuf_images(np.asarray(y, dtype=np.float32), ml_dtypes.bfloat16)
        yi = yi.reshape(NCORES, 128, KL)
        z = np.empty((NCORES, 128, 2 * KL), dtype=ml_dtypes.bfloat16)
        for s, ch in zip(_starts(CHUNKS_BF16), CHUNKS_BF16):
            z[:, :, 2 * s * 128:(2 * s + ch) * 128] = xi[:, :, s * 128:(s + ch) * 128]
            z[:, :, (2 * s + ch) * 128:(2 * s + 2 * ch) * 128] = (
                yi[:, :, s * 128:(s + ch) * 128]
            )
    else:
        npart = NP if impl == "fp8dr124" else 128
        chunks = CHUNKS_FP8_124 if impl == "fp8dr124" else CHUNKS_FP8
        xi = _sbuf_images(xf, ml_dtypes.float8_e4m3, npart)
        yi = _sbuf_images(
            np.asarray(y, dtype=np.float32), ml_dtypes.float8_e4m3, npart
        )
        z = _packed_images(xi, yi, chunks)

    in_maps = [{"zt": np.ascontiguousarray(z[c])} for c in range(NCORES)]

    tmpdir = _os.environ.get("KERNEL_PROFILE_DIR") or None
    res = bass_utils.run_bass_kernel_spmd(
        nc, in_maps, core_ids=list(range(NCORES)), tmpdir=tmpdir
    )
    LAST_RESULTS = res

    acc = np.zeros((N1, N2), dtype=np.float64)
    for c in range(NCORES):
        acc += res.results[c]["out"].astype(np.float64)
    return (acc * scale + VAR_BIAS).astype(np.float32)


# revision 12
# speedup vs baseline: 1.9183x; 1.9183x over previous
"""Trainium2 Bass kernel for nn_Conv2d_35407710388668.

Math: the reference's einsum("icwh,jcwh->ijwh", x, y)/C followed by a
full-spatial VALID box conv collapses to a single GEMM:

    out[i, j] = (1/C) * sum_{c,w,h} x[i,c,w,h] * y[j,c,w,h] * kern[w,h] + 0.1

with contraction K = C*W*H = 131072, M = N = 128.

Sharding: contraction (channel) dim split across the 8 NeuronCores (64
channels each) -- each core reads only its 1/8 slice of BOTH x and y
(total HBM traffic = inputs read exactly once, which is the floor; the
hinted N1-sharding would replicate y 8x).  Each core computes a partial
[128,128] GEMM; the host sums the 8 partials in f64, scales, adds bias.

Default implementation streams the operands as fp8 e4m3 (halves HBM
traffic vs bf16 -- the stream is the roofline here: ~4.2 MB/core at
~336 B/ns) and contracts with DoubleRow fp8 matmuls (2 k-tiles = 256
contraction rows per PE instruction, ~1.4x bf16 PE throughput, so PE
stays under the DMA).  x and y chunks are packed interleaved in ONE
DRAM image so each chunk-pair is a single DMA; chunks alternate between
the SP and ACT HWDGE rings with tapered sizes so PE starts early and
the post-last-chunk tail is short.

fp8 e4m3 is safe here: the output is 0.1 +- ~0.003 and quantization
noise averages out over the 131072-term dot product (measured ~1e-3
relative error, vs the 2e-2 gate).  The conv kernel is folded into x
normalized by 256 (exactly 1.0/elem for the reference's box kernel) so
x stays ~N(0,1) inside fp8 range; the 1/(C*256) scale and +0.1 bias are
applied on host.  |x| <= ~6 << 240, so OCP e4m3 bit patterns match TRN
float8e4 exactly.

Set KERNEL_IMPL=packed for the older bf16 variant, fp8 for fp8 without
DoubleRow.
"""

import numpy as np
import ml_dtypes


def _ensure_axon_profile_hook():
    """Best-effort: register the NTFF profile hook registry that
    concourse.bass_utils expects under axon when trace is requested.
    The container's antenv package lacks the axon_hooks module; the
    actual ctypes hook implementation ships in trn_agent_boot."""
    import sys
    import types

    try:
        import antenv

        if "antenv.axon_hooks" in sys.modules:
            return
        mod = types.ModuleType("antenv.axon_hooks")
        _state = {"hook": None}
        mod.set_axon_ntff_profile_hook = lambda h: _state.__setitem__("hook", h)
        mod.get_axon_ntff_profile_hook = lambda: _state["hook"]
        sys.modules["antenv.axon_hooks"] = mod
        antenv.axon_hooks = mod
        from trn_agent_boot.trn_boot import _ntff_profile_via_ctypes

        mod.set_axon_ntff_profile_hook(
            _ntff_profile_via_ctypes("/opt/axon/libaxon_pjrt.so")
        )
    except Exception:
        pass


_ensure_axon_profile_hook()

N1 = 128
N2 = 128
C = 512
W = 16
H = 16
NCORES = 8
CPC = C // NCORES        # channels per core = 64
KL = CPC * W * H         # per-core contraction length = 16384
KT = KL // 128           # k-tiles per core = 128
VAR_BIAS = 0.1

_CACHE = {}
LAST_RESULTS = None      # test harness reads exec_time_ns from here

# Chunk sizes in k-tiles (one k-tile = 128 contraction rows).  Tapered:
# small first chunks so PE starts early, big middle chunks for DMA
# efficiency, smaller last chunk so the PE tail after the final arrival
# is short.  All even so DoubleRow 2-ktile pairs never straddle a chunk.
CHUNKS_BF16 = [4, 8, 16, 24, 32, 24, 12, 8]
# PE consumes ~2x faster than DMA supplies, so PE-end = land(last chunk)
# + PE(last chunk).  Symmetric pairs keep the SP ring (even chunks) and
# ACT ring (odd chunks) in lockstep with PE consumption order at every
# prefix -- each ring carries [4,8,12,16,12,8,4] -- and the small tail
# chunks keep the post-last-arrival PE tail short.  (Semaphore count
# was measured to NOT affect the fixed ~7us runtime preamble.)
CHUNKS_FP8 = [4, 4, 8, 8, 12, 12, 16, 16, 12, 12, 8, 8, 4, 4]
# 124-row k-tiles: SDMA engine 15 (which serves partitions 92-95 and
# 124-127, and runs ~30% slower than engines 0-14 -- it paces the whole
# stream at 128 rows) gets half an engine's share when rows 124-127 are
# never touched.  16384 real contraction rows pad to 134 tiles x 124.
NP = 124
KTP = 134
CHUNKS_FP8_124 = [4, 8, 16, 24, 24, 18, 14, 12, 8, 6]
assert sum(CHUNKS_BF16) == KT and sum(CHUNKS_FP8) == KT
assert sum(CHUNKS_FP8_124) == KTP and all(c % 2 == 0 for c in CHUNKS_FP8_124)


def _starts(chunks):
    return [sum(chunks[:i]) for i in range(len(chunks))]


def _build_bass_packed():
    """bf16 variant (the previous baseline): x and y chunks packed
    interleaved in ONE DRAM image, chunks alternating between the SP and
    ACT HWDGE rings, 128 accumulating bf16 matmuls."""
    import concourse.bass as bass
    import concourse.mybir as mybir

    CHUNKS = CHUNKS_BF16
    STARTS = _starts(CHUNKS)
    NCHK = len(CHUNKS)

    nc = bass.Bass(
        "TRN2", target_bir_lowering=False, debug=False, num_devices=NCORES
    )
    zt = nc.dram_tensor("zt", [128, 2 * KL], mybir.dt.bfloat16, kind="ExternalInput")
    out = nc.dram_tensor("out", [128, 128], mybir.dt.float32, kind="ExternalOutput")

    zbuf = nc.alloc_sbuf_tensor("zbuf", [128, 2 * KL], mybir.dt.bfloat16)
    rbuf = nc.alloc_sbuf_tensor("rbuf", [128, 128], mybir.dt.float32)
    acc = nc.alloc_psum_tensor("acc", [128, 128], mybir.dt.float32)

    def off_x(c):
        return 2 * STARTS[c] * 128

    def off_y(c):
        return off_x(c) + CHUNKS[c] * 128

    import contextlib

    with contextlib.ExitStack() as st:
        csems = [st.enter_context(nc.semaphore(f"cs{i}")) for i in range(NCHK)]
        ms = st.enter_context(nc.semaphore("ms"))
        vs = st.enter_context(nc.semaphore("vs"))
        osem = st.enter_context(nc.semaphore("osem"))
        blk = st.enter_context(contextlib.ExitStack())
        block = blk.enter_context(nc.Block())

        @block.sync
        def _(sync):
            for c in range(0, NCHK, 2):
                s = slice(off_x(c), off_x(c) + 2 * CHUNKS[c] * 128)
                sync.dma_start(zbuf[:, s], zt[:, s]).then_inc(csems[c], 16)
            sync.wait_ge(vs, 1)
            sync.dma_start(out[:], rbuf[:]).then_inc(osem, 16)
            sync.wait_ge(osem, 16)

        @block.scalar
        def _(scalar):
            for c in range(1, NCHK, 2):
                s = slice(off_x(c), off_x(c) + 2 * CHUNKS[c] * 128)
                scalar.dma_start(zbuf[:, s], zt[:, s]).then_inc(csems[c], 16)

        @block.tensor
        def _(tensor):
            t = 0
            for c in range(NCHK):
                tensor.wait_ge(csems[c], 16)
                for tl in range(CHUNKS[c]):
                    mm = tensor.matmul(
                        acc[:],
                        zbuf[:, off_x(c) + tl * 128:off_x(c) + (tl + 1) * 128],
                        zbuf[:, off_y(c) + tl * 128:off_y(c) + (tl + 1) * 128],
                        start=(t == 0),
                        stop=(t == KT - 1),
                    )
                    t += 1
            mm.then_inc(ms)

        @block.vector
        def _(vector):
            vector.wait_ge(ms, 1)
            vector.tensor_copy(rbuf[:], acc[:]).then_inc(vs)

        blk.close()

    return nc


def _build_bass_fp8(double_row=True, npart=128):
    """fp8 e4m3 operands, optionally DoubleRow (2 k-tiles per PE
    instruction).  The z image is 3D [npart, 2*nkt, 128] so a DoubleRow
    lhsT/rhs AP is a plain [:, 2t:2t+2, :] slice."""
    import concourse.bass as bass
    import concourse.mybir as mybir

    CHUNKS = CHUNKS_FP8 if npart == 128 else CHUNKS_FP8_124
    nkt = KT if npart == 128 else KTP
    STARTS = _starts(CHUNKS)
    NCHK = len(CHUNKS)

    nc = bass.Bass(
        "TRN2", target_bir_lowering=False, debug=False, num_devices=NCORES
    )
    zt = nc.dram_tensor(
        "zt", [npart, 2 * nkt, 128], mybir.dt.float8e4, kind="ExternalInput"
    )
    out = nc.dram_tensor("out", [128, 128], mybir.dt.bfloat16, kind="ExternalOutput")

    zbuf = nc.alloc_sbuf_tensor("zbuf", [npart, 2 * nkt, 128], mybir.dt.float8e4)
    rbuf = nc.alloc_sbuf_tensor("rbuf", [128, 128], mybir.dt.bfloat16)
    acc = nc.alloc_psum_tensor("acc", [128, 128], mybir.dt.float32)

    def off_x(c):  # packed k-tile index of chunk c's x block
        return 2 * STARTS[c]

    def off_y(c):
        return off_x(c) + CHUNKS[c]

    import contextlib

    with contextlib.ExitStack() as st:
        csems = [st.enter_context(nc.semaphore(f"cs{i}")) for i in range(NCHK)]
        ms = st.enter_context(nc.semaphore("ms"))
        vs = st.enter_context(nc.semaphore("vs"))
        osem = st.enter_context(nc.semaphore("osem"))
        blk = st.enter_context(contextlib.ExitStack())
        block = blk.enter_context(nc.Block())

        @block.sync
        def _(sync):
            for c in range(0, NCHK, 2):
                s = slice(off_x(c), off_x(c) + 2 * CHUNKS[c])
                sync.dma_start(zbuf[:, s, :], zt[:, s, :]).then_inc(csems[c], 16)
            sync.wait_ge(vs, 1)
            sync.dma_start(out[:, 0:64], rbuf[:, 0:64]).then_inc(osem, 16)
            sync.wait_ge(osem, 32)

        @block.scalar
        def _(scalar):
            for c in range(1, NCHK, 2):
                s = slice(off_x(c), off_x(c) + 2 * CHUNKS[c])
                scalar.dma_start(zbuf[:, s, :], zt[:, s, :]).then_inc(csems[c], 16)
            scalar.wait_ge(vs, 1)
            scalar.dma_start(out[:, 64:128], rbuf[:, 64:128]).then_inc(osem, 16)

        @block.tensor
        def _(tensor):
            step = 2 if double_row else 1
            pm = mybir.MatmulPerfMode.DoubleRow if double_row else None
            t = 0
            for c in range(NCHK):
                tensor.wait_ge(csems[c], 16)
                for tl in range(0, CHUNKS[c], step):
                    xs = slice(off_x(c) + tl, off_x(c) + tl + step)
                    ys = slice(off_y(c) + tl, off_y(c) + tl + step)
                    mm = tensor.matmul(
                        acc[:],
                        zbuf[:, xs, :],
                        zbuf[:, ys, :],
                        start=(t == 0),
                        stop=(t + step >= nkt),
                        perf_mode=pm,
                    )
                    t += step
            mm.then_inc(ms)

        @block.vector
        def _(vector):
            vector.wait_ge(ms, 1)
            vector.tensor_copy(rbuf[:], acc[:]).then_inc(vs)

        blk.close()

    return nc


def _sbuf_images(a, np_dt, npart=128):
    """[N, C, W, H] -> [core, p, t, m] images; contraction element
    k = t*npart + p of core c maps to input index (c*CPC*256 + k),
    zero-padded past KL when npart*ntiles > KL."""
    nt = KT if npart == 128 else KTP
    b = a.astype(np_dt).reshape(N1, NCORES, KL)
    if npart * nt != KL:
        pad = np.zeros((N1, NCORES, npart * nt - KL), dtype=np_dt)
        b = np.concatenate([b, pad], axis=2)
    b = b.reshape(N1, NCORES, nt, npart).transpose(1, 3, 2, 0)
    return np.ascontiguousarray(b)  # [NCORES, npart, nt, 128]


def _packed_images(xi, yi, chunks):
    """Interleave per-core x/y images chunkwise into one z image
    [NCORES, npart, 2*ntiles, 128]."""
    starts = _starts(chunks)
    nc_, npart, nt, m = xi.shape
    z = np.empty((nc_, npart, 2 * nt, m), dtype=xi.dtype)
    for s, ch in zip(starts, chunks):
        z[:, :, 2 * s:2 * s + ch] = xi[:, :, s:s + ch]
        z[:, :, 2 * s + ch:2 * s + 2 * ch] = yi[:, :, s:s + ch]
    return z


def kernel(x, y, kernel):
    global LAST_RESULTS
    from concourse import bass_utils

    import os as _os

    impl = _os.environ.get("KERNEL_IMPL", "fp8dr")
    if "nc" not in _CACHE:
        _CACHE["nc"] = {
            "packed": _build_bass_packed,
            "fp8": lambda: _build_bass_fp8(double_row=False),
            "fp8dr": lambda: _build_bass_fp8(double_row=True),
            "fp8dr124": lambda: _build_bass_fp8(double_row=True, npart=NP),
        }[impl]()
        _CACHE["impl"] = impl
    nc = _CACHE["nc"]
    impl = _CACHE["impl"]

    k2d = np.asarray(kernel, dtype=np.float32).reshape(W, H)
    xf = np.asarray(x, dtype=np.float32) * (k2d * (W * H))  # ~1.0/elem box kernel
    scale = 1.0 / (C * W * H)

    if impl == "packed":
        xi = _sbuf_images(xf, ml_dtypes.bfloat16).reshape(NCORES, 128, KL)
        yi = _sbuf_images(np.asarray(y, dtype=np.float32), ml_dtypes.bfloat16)
        yi = yi.reshape(NCORES, 128, KL)
        z = np.empty((NCORES, 128, 2 * KL), dtype=ml_dtypes.bfloat16)
        for s, ch in zip(_starts(CHUNKS_BF16), CHUNKS_BF16):
            z[:, :, 2 * s * 128:(2 * s + ch) * 128] = xi[:, :, s * 128:(s + ch) * 128]
            z[:, :, (2 * s + ch) * 128:(2 * s + 2 * ch) * 128] = (
                yi[:, :, s * 128:(s + ch) * 128]
            )
    else:
        npart = NP if impl == "fp8dr124" else 128
        chunks = CHUNKS_FP8_124 if impl == "fp8dr124" else CHUNKS_FP8
        xi = _sbuf_images(xf, ml_dtypes.float8_e4m3, npart)
        yi = _sbuf_images(
            np.asarray(y, dtype=np.float32), ml_dtypes.float8_e4m3, npart
        )
        z = _packed_images(xi, yi, chunks)

    in_maps = [{"zt": np.ascontiguousarray(z[c])} for c in range(NCORES)]

    tmpdir = _os.environ.get("KERNEL_PROFILE_DIR") or None
    res = bass_utils.run_bass_kernel_spmd(
        nc, in_maps, core_ids=list(range(NCORES)), tmpdir=tmpdir
    )
    LAST_RESULTS = res

    acc = np.zeros((N1, N2), dtype=np.float64)
    for c in range(NCORES):
        acc += res.results[c]["out"].astype(np.float64)
    return (acc * scale + VAR_BIAS).astype(np.float32)


# revision 14
# speedup vs baseline: 1.9636x; 1.0236x over previous
"""Trainium2 Bass kernel for nn_Conv2d_35407710388668.

Math: the reference's einsum("icwh,jcwh->ijwh", x, y)/C followed by a
full-spatial VALID box conv collapses to a single GEMM:

    out[i, j] = (1/C) * sum_{c,w,h} x[i,c,w,h] * y[j,c,w,h] * kern[w,h] + 0.1

with contraction K = C*W*H = 131072, M = N = 128.

Sharding: contraction (channel) dim split across the 8 NeuronCores (64
channels each) -- each core reads only its 1/8 slice of BOTH x and y
(total HBM traffic = inputs read exactly once, which is the floor; the
hinted N1-sharding would replicate y 8x).  Each core computes a partial
[128,128] GEMM; the host sums the 8 partials in f64, scales, adds bias.

Default implementation streams the operands as fp8 e4m3 (halves HBM
traffic vs bf16 -- the stream is the roofline here: ~4.2 MB/core at
~336 B/ns) and contracts with DoubleRow fp8 matmuls (2 k-tiles = 256
contraction rows per PE instruction, ~1.4x bf16 PE throughput, so PE
stays under the DMA).  x and y chunks are packed interleaved in ONE
DRAM image so each chunk-pair is a single DMA; chunks alternate between
the SP and ACT HWDGE rings with tapered sizes so PE starts early and
the post-last-chunk tail is short.

fp8 e4m3 is safe here: the output is 0.1 +- ~0.003 and quantization
noise averages out over the 131072-term dot product (measured ~1e-3
relative error, vs the 2e-2 gate).  The conv kernel is folded into x
normalized by 256 (exactly 1.0/elem for the reference's box kernel) so
x stays ~N(0,1) inside fp8 range; the 1/(C*256) scale and +0.1 bias are
applied on host.  |x| <= ~6 << 240, so OCP e4m3 bit patterns match TRN
float8e4 exactly.

Set KERNEL_IMPL=packed for the older bf16 variant, fp8 for fp8 without
DoubleRow.
"""

import numpy as np
import ml_dtypes


def _ensure_axon_profile_hook():
    """Best-effort: register the NTFF profile hook registry that
    concourse.bass_utils expects under axon when trace is requested.
    The container's antenv package lacks the axon_hooks module; the
    actual ctypes hook implementation ships in trn_agent_boot."""
    import sys
    import types

    try:
        import antenv

        if "antenv.axon_hooks" in sys.modules:
            return
        mod = types.ModuleType("antenv.axon_hooks")
        _state = {"hook": None}
        mod.set_axon_ntff_profile_hook = lambda h: _state.__setitem__("hook", h)
        mod.get_axon_ntff_profile_hook = lambda: _state["hook"]
        sys.modules["antenv.axon_hooks"] = mod
        antenv.axon_hooks = mod
        from trn_agent_boot.trn_boot import _ntff_profile_via_ctypes

        mod.set_axon_ntff_profile_hook(
            _ntff_profile_via_ctypes("/opt/axon/libaxon_pjrt.so")
        )
    except Exception:
        pass


_ensure_axon_profile_hook()

N1 = 128
N2 = 128
C = 512
W = 16
H = 16
NCORES = 8
CPC = C // NCORES        # channels per core = 64
KL = CPC * W * H         # per-core contraction length = 16384
KT = KL // 128           # k-tiles per core = 128
VAR_BIAS = 0.1

_CACHE = {}
LAST_RESULTS = None      # test harness reads exec_time_ns from here

# Chunk sizes in k-tiles (one k-tile = 128 contraction rows).  Tapered:
# small first chunks so PE starts early, big middle chunks for DMA
# efficiency, smaller last chunk so the PE tail after the final arrival
# is short.  All even so DoubleRow 2-ktile pairs never straddle a chunk.
CHUNKS_BF16 = [4, 8, 16, 24, 32, 24, 12, 8]
# PE consumes ~2x faster than DMA supplies, so PE-end = land(last chunk)
# + PE(last chunk).  Each ring (SP: even chunks, ACT: odd) carries
# [8,20,24,10,2]: balanced prefixes keep ring delivery in step with PE
# consumption order, the big middle chunks keep DMA packets >=5KB
# (HWDGE descriptor-gen is ~0.6-1.8us per DMA per ring, so many small
# chunks throttle the stream), and the tiny 2-ktile final chunks keep
# the post-last-arrival PE tail short.  (Semaphore count was measured
# to NOT affect the fixed ~7us runtime preamble.)
CHUNKS_FP8 = [8, 8, 20, 20, 24, 24, 10, 10, 2, 2]
# 3-ring variant: chunks round-robin SP / ACT / GpSimd(SWDGE); each
# ring carries [8,20,14,2]=44 or [8,20,12,2]=42 k-tiles.
CHUNKS_FP8_3R = [8, 8, 8, 20, 20, 20, 12, 12, 14, 2, 2, 2]
# 124-row k-tiles: SDMA engine 15 (which serves partitions 92-95 and
# 124-127, and runs ~30% slower than engines 0-14 -- it paces the whole
# stream at 128 rows) gets half an engine's share when rows 124-127 are
# never touched.  16384 real contraction rows pad to 134 tiles x 124.
NP = 124
KTP = 134
CHUNKS_FP8_124 = [4, 8, 16, 24, 24, 18, 14, 12, 8, 6]
assert sum(CHUNKS_BF16) == KT and sum(CHUNKS_FP8) == KT
assert sum(CHUNKS_FP8_124) == KTP and all(c % 2 == 0 for c in CHUNKS_FP8_124)


def _starts(chunks):
    return [sum(chunks[:i]) for i in range(len(chunks))]


def _build_bass_packed():
    """bf16 variant (the previous baseline): x and y chunks packed
    interleaved in ONE DRAM image, chunks alternating between the SP and
    ACT HWDGE rings, 128 accumulating bf16 matmuls."""
    import concourse.bass as bass
    import concourse.mybir as mybir

    CHUNKS = CHUNKS_BF16
    STARTS = _starts(CHUNKS)
    NCHK = len(CHUNKS)

    nc = bass.Bass(
        "TRN2", target_bir_lowering=False, debug=False, num_devices=NCORES
    )
    zt = nc.dram_tensor("zt", [128, 2 * KL], mybir.dt.bfloat16, kind="ExternalInput")
    out = nc.dram_tensor("out", [128, 128], mybir.dt.float32, kind="ExternalOutput")

    zbuf = nc.alloc_sbuf_tensor("zbuf", [128, 2 * KL], mybir.dt.bfloat16)
    rbuf = nc.alloc_sbuf_tensor("rbuf", [128, 128], mybir.dt.float32)
    acc = nc.alloc_psum_tensor("acc", [128, 128], mybir.dt.float32)

    def off_x(c):
        return 2 * STARTS[c] * 128

    def off_y(c):
        return off_x(c) + CHUNKS[c] * 128

    import contextlib

    with contextlib.ExitStack() as st:
        csems = [st.enter_context(nc.semaphore(f"cs{i}")) for i in range(NCHK)]
        ms = st.enter_context(nc.semaphore("ms"))
        vs = st.enter_context(nc.semaphore("vs"))
        osem = st.enter_context(nc.semaphore("osem"))
        blk = st.enter_context(contextlib.ExitStack())
        block = blk.enter_context(nc.Block())

        @block.sync
        def _(sync):
            for c in range(0, NCHK, 2):
                s = slice(off_x(c), off_x(c) + 2 * CHUNKS[c] * 128)
                sync.dma_start(zbuf[:, s], zt[:, s]).then_inc(csems[c], 16)
            sync.wait_ge(vs, 1)
            sync.dma_start(out[:], rbuf[:]).then_inc(osem, 16)
            sync.wait_ge(osem, 16)

        @block.scalar
        def _(scalar):
            for c in range(1, NCHK, 2):
                s = slice(off_x(c), off_x(c) + 2 * CHUNKS[c] * 128)
                scalar.dma_start(zbuf[:, s], zt[:, s]).then_inc(csems[c], 16)

        @block.tensor
        def _(tensor):
            t = 0
            for c in range(NCHK):
                tensor.wait_ge(csems[c], 16)
                for tl in range(CHUNKS[c]):
                    mm = tensor.matmul(
                        acc[:],
                        zbuf[:, off_x(c) + tl * 128:off_x(c) + (tl + 1) * 128],
                        zbuf[:, off_y(c) + tl * 128:off_y(c) + (tl + 1) * 128],
                        start=(t == 0),
                        stop=(t == KT - 1),
                    )
                    t += 1
            mm.then_inc(ms)

        @block.vector
        def _(vector):
            vector.wait_ge(ms, 1)
            vector.tensor_copy(rbuf[:], acc[:]).then_inc(vs)

        blk.close()

    return nc


def _build_bass_fp8(double_row=True, npart=128):
    """fp8 e4m3 operands, optionally DoubleRow (2 k-tiles per PE
    instruction).  The z image is 3D [npart, 2*nkt, 128] so a DoubleRow
    lhsT/rhs AP is a plain [:, 2t:2t+2, :] slice."""
    import concourse.bass as bass
    import concourse.mybir as mybir

    CHUNKS = CHUNKS_FP8 if npart == 128 else CHUNKS_FP8_124
    nkt = KT if npart == 128 else KTP
    STARTS = _starts(CHUNKS)
    NCHK = len(CHUNKS)

    nc = bass.Bass(
        "TRN2", target_bir_lowering=False, debug=False, num_devices=NCORES
    )
    zt = nc.dram_tensor(
        "zt", [npart, 2 * nkt, 128], mybir.dt.float8e4, kind="ExternalInput"
    )
    out = nc.dram_tensor("out", [128, 128], mybir.dt.bfloat16, kind="ExternalOutput")

    zbuf = nc.alloc_sbuf_tensor("zbuf", [npart, 2 * nkt, 128], mybir.dt.float8e4)
    rbuf = nc.alloc_sbuf_tensor("rbuf", [128, 128], mybir.dt.bfloat16)
    acc = nc.alloc_psum_tensor("acc", [128, 128], mybir.dt.float32)

    def off_x(c):  # packed k-tile index of chunk c's x block
        return 2 * STARTS[c]

    def off_y(c):
        return off_x(c) + CHUNKS[c]

    import contextlib

    with contextlib.ExitStack() as st:
        csems = [st.enter_context(nc.semaphore(f"cs{i}")) for i in range(NCHK)]
        ms = st.enter_context(nc.semaphore("ms"))
        vs = st.enter_context(nc.semaphore("vs"))
        osem = st.enter_context(nc.semaphore("osem"))
        blk = st.enter_context(contextlib.ExitStack())
        block = blk.enter_context(nc.Block())

        @block.sync
        def _(sync):
            for c in range(0, NCHK, 2):
                s = slice(off_x(c), off_x(c) + 2 * CHUNKS[c])
                sync.dma_start(zbuf[:, s, :], zt[:, s, :]).then_inc(csems[c], 16)
            sync.wait_ge(vs, 1)
            sync.dma_start(out[:, 0:64], rbuf[:, 0:64]).then_inc(osem, 16)
            sync.wait_ge(osem, 32)

        @block.scalar
        def _(scalar):
            for c in range(1, NCHK, 2):
                s = slice(off_x(c), off_x(c) + 2 * CHUNKS[c])
                scalar.dma_start(zbuf[:, s, :], zt[:, s, :]).then_inc(csems[c], 16)
            scalar.wait_ge(vs, 1)
            scalar.dma_start(out[:, 64:128], rbuf[:, 64:128]).then_inc(osem, 16)

        @block.tensor
        def _(tensor):
            step = 2 if double_row else 1
            pm = mybir.MatmulPerfMode.DoubleRow if double_row else None
            t = 0
            for c in range(NCHK):
                tensor.wait_ge(csems[c], 16)
                for tl in range(0, CHUNKS[c], step):
                    xs = slice(off_x(c) + tl, off_x(c) + tl + step)
                    ys = slice(off_y(c) + tl, off_y(c) + tl + step)
                    mm = tensor.matmul(
                        acc[:],
                        zbuf[:, xs, :],
                        zbuf[:, ys, :],
                        start=(t == 0),
                        stop=(t + step >= nkt),
                        perf_mode=pm,
                    )
                    t += step
            mm.then_inc(ms)

        @block.vector
        def _(vector):
            vector.wait_ge(ms, 1)
            vector.tensor_copy(rbuf[:], acc[:]).then_inc(vs)

        blk.close()

    return nc


def _sbuf_images(a, np_dt, npart=128):
    """[N, C, W, H] -> [core, p, t, m] images; contraction element
    k = t*npart + p of core c maps to input index (c*CPC*256 + k),
    zero-padded past KL when npart*ntiles > KL."""
    nt = KT if npart == 128 else KTP
    b = a.astype(np_dt).reshape(N1, NCORES, KL)
    if npart * nt != KL:
        pad = np.zeros((N1, NCORES, npart * nt - KL), dtype=np_dt)
        b = np.concatenate([b, pad], axis=2)
    b = b.reshape(N1, NCORES, nt, npart).transpose(1, 3, 2, 0)
    return np.ascontiguousarray(b)  # [NCORES, npart, nt, 128]


def _packed_images(xi, yi, chunks):
    """Interleave per-core x/y images chunkwise into one z image
    [NCORES, npart, 2*ntiles, 128]."""
    starts = _starts(chunks)
    nc_, npart, nt, m = xi.shape
    z = np.empty((nc_, npart, 2 * nt, m), dtype=xi.dtype)
    for s, ch in zip(starts, chunks):
        z[:, :, 2 * s:2 * s + ch] = xi[:, :, s:s + ch]
        z[:, :, 2 * s + ch:2 * s + 2 * ch] = yi[:, :, s:s + ch]
    return z


def kernel(x, y, kernel):
    global LAST_RESULTS
    from concourse import bass_utils

    import os as _os

    impl = _os.environ.get("KERNEL_IMPL", "fp8dr")
    if "nc" not in _CACHE:
        _CACHE["nc"] = {
            "packed": _build_bass_packed,
            "fp8": lambda: _build_bass_fp8(double_row=False),
            "fp8dr": lambda: _build_bass_fp8(double_row=True),
            "fp8dr124": lambda: _build_bass_fp8(double_row=True, npart=NP),
        }[impl]()
        _CACHE["impl"] = impl
    nc = _CACHE["nc"]
    impl = _CACHE["impl"]

    k2d = np.asarray(kernel, dtype=np.float32).reshape(W, H)
    xf = np.asarray(x, dtype=np.float32) * (k2d * (W * H))  # ~1.0/elem box kernel
    scale = 1.0 / (C * W * H)

    if impl == "packed":
        xi = _sbuf_images(xf, ml_dtypes.bfloat16).reshape(NCORES, 128, KL)
        yi = _sbuf_images(np.asarray(y, dtype=np.float32), ml_dtypes.bfloat16)
        yi = yi.reshape(NCORES, 128, KL)
        z = np.empty((NCORES, 128, 2 * KL), dtype=ml_dtypes.bfloat16)
        for s, ch in zip(_starts(CHUNKS_BF16), CHUNKS_BF16):
            z[:, :, 2 * s * 128:(2 * s + ch) * 128] = xi[:, :, s * 128:(s + ch) * 128]
            z[:, :, (2 * s + ch) * 128:(2 * s + 2 * ch) * 128] = (
                yi[:, :, s * 128:(s + ch) * 128]
            )
    else:
        npart = NP if impl == "fp8dr124" else 128
        chunks = CHUNKS_FP8_124 if impl == "fp8dr124" else CHUNKS_FP8
        xi = _sbuf_images(xf, ml_dtypes.float8_e4m3, npart)
        yi = _sbuf_images(
            np.asarray(y, dtype=np.float32), ml_dtypes.float8_e4m3, npart
        )
        z = _packed_images(xi, yi, chunks)

    in_maps = [{"zt": np.ascontiguousarray(z[c])} for c in range(NCORES)]

    tmpdir = _os.environ.get("KERNEL_PROFILE_DIR") or None
    res = bass_utils.run_bass_kernel_spmd(
        nc, in_maps, core_ids=list(range(NCORES)), tmpdir=tmpdir
    )
    LAST_RESULTS = res

    acc = np.zeros((N1, N2), dtype=np.float64)
    for c in range(NCORES):
        acc += res.results[c]["out"].astype(np.float64)
    return (acc * scale + VAR_BIAS).astype(np.float32)
